# revision 32
# baseline (speedup 1.0000x reference)
"""Trainium2 Bass kernel for nn_CRAP_16544214024675 (sparse_attention).

Reference computation (per batch b, channel c):
  q = Wq@feat + bq                        (1x1 conv over channels)
  k = unfold3x3_s2(src)                   (strided window gather, pad 1)
  v = unfold3x3_s2(Wv@src + bv)
  A = softmax_t( sum_px q*k_t / 64 )      (9 window positions)
  out = fold3x3_s1( A_t * v_t ) * feat

Sharding: 8 cores = 4 batches x 2 output-channel halves. The per-core
program is identical: inputs are packed so channel-slot 0 is always the
core's OWN half (host reorders the contraction; channel sums commute).

Layout trick: unfold/fold never materialize. With parity planes
P[p,q][c,h,w] = x[c, 2h+p, 2w+q]:
  k_(i,j)[h,w] = src[2h+i-1, 2w+j-1] -> srcplane[(i+1)%2][(j+1)%2]
    shifted by (i==0 ? -1 : 0, j==0 ? -1 : 0)
  fold sample for t=(i,j) at (y,x) = vsrc[2y+1-i, 2x+1-j]
    -> vplane[(i+1)%2][(j+1)%2][y + (i==2 ? -1 : 0), x + (j==2 ? -1 : 0)]
Out-of-range samples are the zero-pad terms; every op restricts to its
valid window so they drop out exactly.

Changes vs the 84us baseline (trace-driven; measured 76.1us):
  * PE p-state: TRN2 PE ramps 0.65->1.2->2.4GHz after ~3us of continuous
    work and drops back after ~2.7us idle. The baseline ran the whole
    kernel at 1.2GHz (matmuls 630ns/512rows). The warm-up burst (26
    512-row dummy matmuls) now covers the DMA head exactly, so convs and
    folds run mostly at 2.4GHz (379ns/512rows). Warmups of 14/18 measured
    WORSE (PE idles pre-featb and restarts at MID).
  * DMA: 18 issues -> 8 (wpack=wq+wv+ident, bpack, featb halves, 4 plane
    pairs); featf dropped (finale reuses bf16 featb slot 0); output in
    bf16 quarters (host converts back to f32). 14.8MB -> 10.2MB.
  * Logits: 6 t's as DVE STT with fused accum (1x); 3 j!=0 t's as DVE
    tensor_mul (2x, even-aligned views) + ACT copy-reduce. Dedicated
    scratch pools per mode so buffer recycling never stalls the DVE
    product stream.
  * Finale split into 16-row quarters pipelined with the output DMA.
  Measured dead ends: vector.tensor_tensor_reduce and gpsimd STT-accum
  crash/fail NEFF compile; gpsimd tensor ops run 3-6x below their cost
  model (Q7 LOAD_LIB swaps between op types) and gpsimd cannot read
  PSUM; 16-row v-conv psum chunks (to co-allocate a fold half-PSUM for
  early fold interleave) starve the PE behind ACT copies.
"""
import contextlib
import sys
from contextlib import ExitStack

import numpy as np


def _nullctx():
    return contextlib.nullcontext()

for _p in ("/opt/trn_rl_repo", "/root/.axon_site/_ro/trn_rl_repo"):
    if _p not in sys.path:
        sys.path.append(_p)

import ml_dtypes

import concourse.tile as tile
from concourse import bacc, mybir
from concourse import bass_utils
from concourse.bass_interp import get_hw_module

F32 = mybir.dt.float32
BF16 = mybir.dt.bfloat16
AF = mybir.ActivationFunctionType
ALU = mybir.AluOpType

B, C, H, W = 4, 256, 64, 64
N_CORES = 8

# Plane (p,q) load order; plane (0,0) hosts the full-window t=(1,1) which
# must fold first (PSUM start=True), so it loads first.
PLANE_ORDER = [(0, 0), (1, 1), (1, 0), (0, 1)]
# t's ordered by hosting plane (pq = ((i+1)%2,(j+1)%2)) so products/folds
# consume planes in arrival order.
T_ORDER = [
    (1, 1),
    (0, 0), (0, 2), (2, 0), (2, 2),
    (0, 1), (2, 1),
    (1, 0), (1, 2),
]
N_WARMUP = 26


def build_program():
    nc = bacc.Bacc("TRN2", target_bir_lowering=False, debug=False)

    # wpack rows: 0=wq[0] 1=wq[1] 2=wv[0] 3=wv[1] 4=ident (lhsT layouts)
    wpack_d = nc.dram_tensor("wpack", (5, 128, 128), BF16, kind="ExternalInput")
    bpack_d = nc.dram_tensor("bpack", (128, 2), F32, kind="ExternalInput")
    featb_d = nc.dram_tensor("featb", (2, 128, H, W), BF16, kind="ExternalInput")
    spl_d = nc.dram_tensor("splanes", (4, 2, 128, H, W), BF16, kind="ExternalInput")
    out_d = nc.dram_tensor("out", (128, H, W), BF16, kind="ExternalOutput")

    with tile.TileContext(nc) as tc, ExitStack() as ctx:
        pool = ctx.enter_context(tc.tile_pool(name="main", bufs=1))
        dgpool = ctx.enter_context(tc.tile_pool(name="diags", bufs=9))

        # --- batched input DMA on sync HWDGE, ordered by first consumer ---
        wpack_t = pool.tile([128, 5, 128], BF16, tag="wpack")
        bpack_t = pool.tile([128, 2], F32, tag="bpack")
        featb_t = pool.tile([128, 2, H, W], BF16, tag="featb")
        nc.sync.dma_start(wpack_t[:], wpack_d.ap().rearrange("a p b -> p a b"))
        nc.sync.dma_start(bpack_t[:], bpack_d.ap())
        # two halves so the q-conv k=0 pass starts ~2.5us earlier
        nc.sync.dma_start(featb_t[:, 0], featb_d.ap()[0])
        nc.sync.dma_start(featb_t[:, 1], featb_d.ap()[1])
        # slot0 (own channels: feeds the logit products AND the v-conv k=0
        # pass) lands ~2.5us before slot1 completes the pair
        spl_t = []
        for pi, (p, q) in enumerate(PLANE_ORDER):
            t_ = pool.tile([128, 2, H, W], BF16, tag=f"spl{p}{q}", name=f"spl{p}{q}")
            nc.sync.dma_start(t_[:, 0], spl_d.ap()[pi, 0])
            nc.sync.dma_start(t_[:, 1], spl_d.ap()[pi, 1])
            spl_t.append(t_)
        splane = {}
        for pi, (p, q) in enumerate(PLANE_ORDER):
            splane[(p, q)] = spl_t[pi]

        with tc.tile_pool(name="psq", bufs=2, space="PSUM") as psq:
            # --- PE warm-up: dependency-free matmuls open the clock gate and
            # ramp the p-state while the head DMAs stream in ---
            warm_t = pool.tile([128, 640], BF16, tag="warm")
            nc.vector.memset(warm_t[:], 0.5)
            wps = psq.tile([128, 32, W], F32, tag="ps", name="warmps")
            for w_i in range(N_WARMUP):
                nc.tensor.matmul(
                    wps[:, 0:8, :],
                    warm_t[:, 0:128],
                    warm_t[:, 128:640],
                    start=True,
                    stop=True,
                    skip_group_check=True,
                )

            # --- q-conv: q = Wq@feat + bq -> bf16 ---
            q_t = pool.tile([128, H, W], BF16, tag="q")
            for half in range(2):
                ps = psq.tile([128, 32, W], F32, tag="ps")
                r0 = 32 * half
                for k in range(2):
                    for s in range(4):
                        nc.tensor.matmul(
                            ps[:, 8 * s : 8 * s + 8, :],
                            wpack_t[:, k, :],
                            featb_t[:, k, r0 + 8 * s : r0 + 8 * s + 8, :],
                            start=(k == 0),
                            stop=(k == 1),
                        )
                with tc.high_priority():
                    nc.scalar.activation(
                        q_t[:, r0 : r0 + 32, :], ps[:],
                        AF.Identity, bias=bpack_t[:, 0:1],
                    )

            # --- v-conv per plane: vplane = Wv@srcplane + bv ---
            vplane = {}
            for (p, q) in PLANE_ORDER:
                vplane[(p, q)] = pool.tile(
                    [128, H, W], BF16, tag=f"vpl{p}{q}", name=f"vpl{p}{q}"
                )
                for half in range(2):
                    ps = psq.tile([128, 32, W], F32, tag="ps")
                    r0 = 32 * half
                    for k in range(2):
                        for s in range(4):
                            nc.tensor.matmul(
                                ps[:, 8 * s : 8 * s + 8, :],
                                wpack_t[:, 2 + k, :],
                                splane[(p, q)][:, k, r0 + 8 * s : r0 + 8 * s + 8, :],
                                start=(k == 0),
                                stop=(k == 1),
                            )
                    with tc.high_priority():
                        nc.scalar.activation(
                            vplane[(p, q)][:, r0 : r0 + 32, :],
                            ps[:],
                            AF.Identity,
                            bias=bpack_t[:, 1:2],
                        )

        # --- logits: q.k_t reductions, split two ways:
        #     'stt'  DVE fused product+accum (1x, single engine; used for
        #            all j==0 t's, which can't meet the 2x alignment rule)
        #     'ttd'  DVE TT product (2x, even-aligned j!=0 views)
        #            + ACT copy-reduce ---
        psf = ctx.enter_context(tc.tile_pool(name="psf", bufs=1, space="PSUM"))
        lg_t = pool.tile([128, 9], F32, tag="lg")
        exp_t = pool.tile([128, 9], F32, tag="exp")
        fold_ps = psf.tile([128, H, W], F32, tag="fold")
        sc_stt = ctx.enter_context(tc.tile_pool(name="scstt", bufs=1))
        sc_ttd = ctx.enter_context(tc.tile_pool(name="scttd", bufs=2))

        # TT t's are picked so their ACT reduces land AFTER the last v-copy:
        # a reduce scheduled mid-v-conv delays the P10/P01 psum drains, which
        # delays fold_ps allocation (needs all 8 banks) and serializes every
        # fold chunk after it.
        T_ASSIGN = {
            (1, 1): "stt", (0, 0): "stt", (2, 0): "stt",
            (0, 2): "stt", (2, 2): "stt", (1, 0): "stt", (1, 2): "stt",
            (0, 1): "ttd", (2, 1): "ttd",
        }
        for idx, (i, j) in enumerate(T_ORDER):
            pq = ((i + 1) % 2, (j + 1) % 2)
            if i == 0:
                qr0, rows = 1, 63
            else:
                qr0, rows = 0, 64
            if j == 0:
                qc0, cols = 1, 63
            else:
                qc0, cols = 0, 64
            pl = splane[pq]
            q_view = q_t[:, qr0 : qr0 + rows, qc0 : qc0 + cols]
            p_view = pl[:, 0, 0:rows, 0:cols]
            mode = T_ASSIGN[(i, j)]
            scp = {"stt": sc_stt, "ttd": sc_ttd}[mode]
            sc = scp.tile([128, H, W], BF16, tag=f"prod_{mode}", name=f"prod{idx}")
            if mode == "stt":
                with tc.high_priority():
                    nc.vector.scalar_tensor_tensor(
                        out=sc[:, 0:rows, 0:cols],
                        in0=q_view,
                        scalar=1.0,
                        in1=p_view,
                        op0=ALU.mult,
                        op1=ALU.mult,
                        accum_out=lg_t[:, idx : idx + 1],
                    )
            else:
                with tc.high_priority():
                    nc.vector.tensor_mul(sc[:, 0:rows, 0:cols], q_view, p_view)
                nc.scalar.activation(
                    sc[:, 0:rows, 0:cols],
                    sc[:, 0:rows, 0:cols],
                    AF.Copy,
                    accum_out=lg_t[:, idx : idx + 1],
                )
            prio = tc.high_priority() if mode == "stt" else _nullctx()
            with prio:
                nc.scalar.activation(
                    exp_t[:, idx : idx + 1],
                    lg_t[:, idx : idx + 1],
                    AF.Exp,
                    scale=1.0 / 64.0,
                )
            dg = dgpool.tile([128, 128], BF16, tag="diag", name=f"diag{idx}")
            with tc.high_priority():
                nc.scalar.activation(
                    dg[:], wpack_t[:, 4, :], AF.Identity,
                    scale=exp_t[:, idx : idx + 1],
                )

            # fold windows: psum[y,x] += exp_t * vplane[pq][y+dy, x+dx]
            if i == 0:
                yo0, yo1, dy = 0, 63, 0
            elif i == 1:
                yo0, yo1, dy = 0, 64, 0
            else:
                yo0, yo1, dy = 1, 64, -1
            if j == 0:
                xo0, xo1, dx = 0, 63, 0
            elif j == 1:
                xo0, xo1, dx = 0, 64, 0
            else:
                xo0, xo1, dx = 1, 64, -1
            vp = vplane[pq]
            yb = yo0
            while yb < yo1:
                ye = min((yb // 8 + 1) * 8, yo1)
                nc.tensor.matmul(
                    fold_ps[:, yb:ye, xo0:xo1],
                    dg[:],
                    vp[:, yb + dy : ye + dy, xo0 + dx : xo1 + dx],
                    start=(idx == 0),
                    stop=(idx == 8),
                    skip_group_check=True,
                )
                yb = ye

        # --- 1/Z off the critical path ---
        z_t = pool.tile([128, 1], F32, tag="z")
        rz_t = pool.tile([128, 1], F32, tag="rz")
        nc.vector.tensor_reduce(z_t[:], exp_t[:], axis=mybir.AxisListType.X, op=ALU.add)
        nc.vector.reciprocal(rz_t[:], z_t[:])

        # --- final: out = (fold * 1/Z) * feat, four quarters pipelined with
        #     the output DMA (and with the tail of the last fold t) ---
        out_t = pool.tile([128, H, W], BF16, tag="out")
        for qt in range(4):
            r0 = 16 * qt
            nc.vector.scalar_tensor_tensor(
                out=out_t[:, r0 : r0 + 16, :],
                in0=fold_ps[:, r0 : r0 + 16, :],
                scalar=rz_t[:],
                in1=featb_t[:, 0, r0 : r0 + 16, :],
                op0=ALU.mult,
                op1=ALU.mult,
            )
            nc.sync.dma_start(out_d.ap()[:, r0 : r0 + 16, :], out_t[:, r0 : r0 + 16, :])

    nc.compile()
    nc.m = get_hw_module(nc.m)
    return nc


_PROGRAM = None


def _get_program():
    global _PROGRAM
    if _PROGRAM is None:
        _PROGRAM = build_program()
    return _PROGRAM


def _prep_inputs(feat, src, Wq, bq, Wv, bv):
    bf = ml_dtypes.bfloat16
    # src parity planes: (B, ct, p, q, 128, H, W)
    spl = np.ascontiguousarray(
        src.reshape(B, 2, 128, H, 2, W, 2).transpose(0, 1, 4, 6, 2, 3, 5)
    ).astype(bf)
    featb = feat.reshape(B, 2, 128, H, W).astype(bf)
    identb = np.eye(128, dtype=np.float32)
    # lhsT layout [ct_in, cin_local, cout]: Wq.T[cin, cout] split over cin
    wq3 = np.ascontiguousarray(Wq.T).reshape(2, 128, C)
    wv3 = np.ascontiguousarray(Wv.T).reshape(2, 128, C)
    in_maps = []
    for core in range(N_CORES):
        b, h = divmod(core, 2)
        oc = slice(h * 128, h * 128 + 128)
        order = [h, 1 - h]  # slot 0 = own input-channel half
        wpack = np.empty((5, 128, 128), np.float32)
        wpack[0:2] = wq3[order][:, :, oc]
        wpack[2:4] = wv3[order][:, :, oc]
        wpack[4] = identb
        bpack = np.stack([bq[oc], bv[oc]], axis=1).astype(np.float32)
        # planes in PLANE_ORDER, each (ct, 128, H, W)
        spl_core = np.stack(
            [spl[b][order][:, p, q] for (p, q) in PLANE_ORDER], axis=0
        )
        in_maps.append(
            dict(
                wpack=wpack.astype(bf),
                bpack=np.ascontiguousarray(bpack),
                featb=np.ascontiguousarray(featb[b][order]),
                splanes=np.ascontiguousarray(spl_core),
            )
        )
    return in_maps


def kernel(feat, src, Wq, bq, Wv, bv, _trace=False):
    feat = np.asarray(feat, np.float32)
    src = np.asarray(src, np.float32)
    Wq = np.asarray(Wq, np.float32)
    bq = np.asarray(bq, np.float32)
    Wv = np.asarray(Wv, np.float32)
    bv = np.asarray(bv, np.float32)

    in_maps = _prep_inputs(feat, src, Wq, bq, Wv, bv)
    nc = _get_program()
    res = bass_utils.run_bass_kernel_spmd(
        nc, in_maps, core_ids=list(range(N_CORES)), trace=_trace
    )
    out = np.empty((B, C, H, W), np.float32)
    for core in range(N_CORES):
        b, h = divmod(core, 2)
        out[b, h * 128 : h * 128 + 128] = res.results[core]["out"].astype(np.float32)
    if _trace:
        kernel.last_results = res
    return out


kernel.last_results = None
